# revision 8
# baseline (speedup 1.0000x reference)
"""Euclidean fast attention TRN2 kernel (v3).

Math: the reference computes per-graph linear attention with a 6-point
Lebedev grid (octahedron +-x,+-y,+-z) RoPE. For the +-u pairs the sin
cross-terms cancel, so the pairwise score matrix reduces to

    S[n,n'] = (1/3) sum_axis sum_mf Q[n,mf] K[n',mf] * cos(theta_{f//2} *
              (p[n,axis] - p[n',axis]))
            = (1/3) sum_axis [ (Q.C_a)(K.C_a)^T + (Q.S_a)(K.S_a)^T ]
    out = S @ V        (per graph, n=256 nodes)

with C_a[n,f] = cos(theta_{f//2} p[n,a]), S_a likewise with sin. Since
n (256) << Mdim (1152), this pairwise form is ~6.5x fewer FLOPs than the
reference's KV-summary form. The 1/3 quadrature weight is folded into Wq
host-side; x is pre-transposed to feature-major and the RoPE cos/sin
tables are precomputed on the host (standard rotary-cache practice).

v3 schedule (per core = 512 nodes = 2 graphs):
 - un-duplicated cos/sin tables; stride-0-repeat APs broadcast each
   [128,512] table across m-blocks (saves 2.4MB DMA/core).
 - the DVE owns ALL table modulation (concurrent GpSimd TENSOR_TENSOR
   was measured to crush DVE throughput ~4x via SBUF port contention).
   Its 18 ops run availability-major: (m0-2 x t0..5), (m3-5 x ...),
   (m6-8 x ...), so the first PSUM->SBUF copies unlock a full 6-table
   run of work. PSUM copies are split ACT/GpSimd so the DVE stream
   stays pure modulation.
 - PE score matmuls consume ranges in DVE production order (PSUM
   accumulation is order-free); V-projection groups are interleaved
   between score blocks as filler for the DVE-paced gaps.
 - the framework's initial all-engine barrier is stripped: engines
   start as soon as their ucode loads, letting the SP kick DMAs and the
   PE project while ACT/DVE/Pool are still initializing.
 - output staged bf16 (halves the output DMA); host upcasts to fp32.

Sharding: 8 cores x 2 graphs (512 contiguous nodes) each; no collectives.
"""

import numpy as np

import concourse.bass as bass
from concourse import mybir
from concourse.tile import TileContext
from concourse.bass_utils import run_bass_kernel_spmd

# ---- problem constants (hardcoded per contract) ----
N = 4096
B = 16
NUM_DEG = 9
F = 128
FQK = 128
FV = 128
MAX_FREQ = 8.0
MAX_LEN = 10.0
L = 2
N_CORES = 8
NS = N // N_CORES          # 512 nodes per core
NG = N // B                # 256 nodes per graph
GPC = NS // NG             # 2 graphs per core
MF = NUM_DEG * FQK         # 1152
H = FQK // 2               # 64
MW = 2 * NS                # 1024: per-m width of the [q_m | k_m] block

DEG_IDX = np.array([l for l in range(L + 1) for _ in range(2 * l + 1)], dtype=np.int64)

FP = mybir.dt.float32
BF = mybir.dt.bfloat16


def _split_multi_waits(nc):
    """This env's walrus rejects >1 sync wait per instruction; hoist extras
    onto single-wait NoOps on the same engine, preserving program order."""
    ctr = 0
    for f in nc.m.functions:
        for bb in f.blocks:
            new = []
            for inst in bb.instructions:
                si = inst.sync_info
                if si is not None and len(si.on_wait) > 1:
                    waits = list(si.on_wait)
                    for w in waits[:-1]:
                        ctr += 1
                        new.append(mybir.InstNoOp(
                            name=f"wsplit-{ctr}",
                            engine=inst.engine,
                            sync_info=mybir.SyncInfo(on_wait=[w], on_update=[]),
                        ))
                    si.on_wait = waits[-1:]
                    inst.sync_info = si
                new.append(inst)
            bb.instructions[:] = new


class _CompatTC(TileContext):
    def _drain_and_barrier(self, tick_clock, wait_clock):
        # Lean ending: per-sem single-wait drains (this walrus allows only one
        # sync wait per instruction), one barrier, sem clears for safe
        # re-execution. The stock version adds a second barrier (~3.5us).
        from concourse.vector_clock import ScopedClock
        drain_inst = self.nc.sync.drain()
        wait_clock.add_sem_waits(drain_inst.ins,
                                 ScopedClock({None: tick_clock.global_clock}))
        inst = drain_inst.ins
        si = inst.sync_info
        waits = list(si.on_wait) if si is not None else []
        if len(waits) > 1:
            si.on_wait = waits[:1]
            inst.sync_info = si
            for w in waits[1:]:
                d2 = self.nc.sync.drain()
                d2.ins.sync_info = type(si)(on_wait=[w], on_update=[])
        hs = self.nc.alloc_semaphore("tail_handshake")
        self.nc.sync.sem_inc(hs, 1)
        self.nc.gpsimd.wait_ge(hs, 1)
        popped = self.nc._tile_sem_poison_stack.pop()
        assert popped is self._sem_poison
        self.nc.clear_and_free_semaphores(
            list(self.sems.allocated().values()) + [hs])

    def __exit__(self, *args):
        r = super().__exit__(*args)
        if args[0] is None:
            _split_multi_waits(self.nc)
        return r


def _strip_preamble(nc):
    """Drop the framework's const-AP memsets, the initial all-engine
    barrier, and leading global sync junk: engines start independently
    (all sems count up from 0; the tail handshake restores them)."""
    f = nc.m.functions[0]
    bb = f.blocks[0]
    out = []
    for inst in bb.instructions:
        tname = type(inst).__name__
        if tname == 'InstMemset' and inst.outs and 'const-' in str(inst.outs[0]):
            continue
        if tname in ('InstDrain', 'InstEventSemaphore') and \
                'barrier' in str(getattr(inst, 'name', '')) + str(inst.sync_info):
            continue
        out.append(inst)
    pruned = []
    seen_real = False
    for inst in out:
        tname = type(inst).__name__
        if not seen_real and tname in ('InstEventSemaphore', 'InstDrain', 'InstMemset', 'InstNoOp'):
            if tname == 'InstMemset':
                pruned.append(inst)
            continue
        seen_real = True
        pruned.append(inst)
    bb.instructions[:] = pruned


def _rep(tile_ap, nrep):
    """AP reading tile_ap's [128, W] block repeated nrep times along the
    free dim (stride-0 middle dim)."""
    return bass.AP(tensor=tile_ap.tensor, offset=tile_ap.offset,
                   ap=[tile_ap.ap[0], [0, nrep], tile_ap.ap[1]])


def _build(with_bias):
    nc = bass.Bass("TRN2")
    xtb = nc.dram_tensor("xtb", [MF, NS], BF, kind="ExternalInput")
    tabs = nc.dram_tensor("tabs", [6 * 128, NS], BF, kind="ExternalInput")
    wq = nc.dram_tensor("wq", [128, 3 * 128], BF, kind="ExternalInput")
    wk = nc.dram_tensor("wk", [128, 3 * 128], BF, kind="ExternalInput")
    wv = nc.dram_tensor("wv", [128, 3 * 128], BF, kind="ExternalInput")
    if with_bias:
        bq = nc.dram_tensor("bq", [128, 1], FP, kind="ExternalInput")
        bk = nc.dram_tensor("bk", [128, 1], FP, kind="ExternalInput")
        bvr = nc.dram_tensor("bvr", [1, 128], FP, kind="ExternalInput")
    y = nc.dram_tensor("y", [NS, MF], BF, kind="ExternalOutput")

    with _CompatTC(nc) as tc:
        _emit(nc, tc, locals(), with_bias)
    _strip_preamble(nc)
    return nc


# modulation ranges, availability-major: each (lo, hi) m-range is produced
# for all 6 tables before moving to the next range. m0 alone first so the
# DVE stream starts as soon as the first two PSUM copies land.
RANGES = [(0, 1), (1, 3), (3, 6), (6, 9)]


def _emit(nc, tc, T, with_bias):
    xtb, tabs, wq, wk, wv, y = (
        T["xtb"], T["tabs"], T["wq"], T["wk"], T["wv"], T["y"])

    from contextlib import ExitStack
    ctx = ExitStack()
    with ctx:
        const = ctx.enter_context(tc.tile_pool(name="const", bufs=1))
        feats = ctx.enter_context(tc.tile_pool(name="feats", bufs=1))
        stp = ctx.enter_context(tc.tile_pool(name="stp", bufs=1))
        outp = ctx.enter_context(tc.tile_pool(name="outp", bufs=1))
        psQ = ctx.enter_context(tc.tile_pool(name="psQ", bufs=3, space="PSUM"))
        psV = ctx.enter_context(tc.tile_pool(name="psV", bufs=3, space="PSUM"))
        psS = ctx.enter_context(tc.tile_pool(name="psS", bufs=1, space="PSUM"))

        # ---- input DMAs: split across the two HWDGE queues (SP + ACT) so
        # transfers run concurrently; order within each queue = need order.
        wq_t = const.tile([128, 384], BF)
        nc.sync.dma_start(out=wq_t, in_=wq[:, :])
        xT = [feats.tile([128, NS], BF, name=f"xT{m}") for m in range(NUM_DEG)]
        for m in range(NUM_DEG):
            nc.sync.dma_start(out=xT[m], in_=xtb[m * 128:(m + 1) * 128, :])
        wk_t = const.tile([128, 384], BF)
        nc.scalar.dma_start(out=wk_t, in_=wk[:, :])
        tab = [feats.tile([128, NS], BF, name=f"tab{i}") for i in range(6)]
        for i in range(6):
            nc.scalar.dma_start(out=tab[i], in_=tabs[i * 128:(i + 1) * 128, :])
        wv_t = const.tile([128, 384], BF)
        nc.scalar.dma_start(out=wv_t, in_=wv[:, :])
        if with_bias:
            bq_t = const.tile([128, 1], FP)
            nc.sync.dma_start(out=bq_t, in_=T["bq"][:, :])
            bk_t = const.tile([128, 1], FP)
            nc.sync.dma_start(out=bk_t, in_=T["bk"][:, :])
            bvb = const.tile([128, 128], FP)
            nc.sync.dma_start(out=bvb, in_=bass.AP(
                tensor=T["bvr"].ap().tensor, offset=0, ap=[[0, 128], [1, 128]]))

        # ---- Q/K projections -> qkt [q_m | k_m] x 9; copies ACT/Pool ----
        qkt = feats.tile([128, NUM_DEG * MW], BF, name="qkt")
        copy_jobs = []     # (psum, dst_lo, dst_hi, engine_idx) alternating
        for m in range(NUM_DEG):
            d = int(DEG_IDX[m])
            pq = psQ.tile([128, 512], FP, name="psq")
            nc.tensor.matmul(pq, wq_t[:, d * 128:(d + 1) * 128], xT[m],
                             start=True, stop=True)
            pk = psQ.tile([128, 512], FP, name="psq")
            nc.tensor.matmul(pk, wk_t[:, d * 128:(d + 1) * 128], xT[m],
                             start=True, stop=True)
            copy_jobs.append((m, pq, pk))

        # all QK copies on ACT, in m order (GpSimd cannot read PSUM)
        for m, pq, pk in copy_jobs:
            if with_bias and m == 0:
                nc.vector.tensor_scalar_add(qkt[:, 0:NS], pq, bq_t)
                nc.vector.tensor_scalar_add(qkt[:, NS:MW], pk, bk_t)
                continue
            nc.scalar.copy(qkt[:, m * MW:m * MW + NS], pq)
            nc.scalar.copy(qkt[:, m * MW + NS:(m + 1) * MW], pk)

        # ---- DVE: all table modulation, availability-major ----
        qc = [feats.tile([128, NUM_DEG * MW], BF, name=f"qc{t}") for t in range(6)]
        dve_sched = []
        for (a, b) in RANGES:
            for t in range(6):
                if (a, b) == RANGES[-1] and t == 5:
                    # finest granularity at the very end: sharper PE tail
                    for m in range(a, b):
                        dve_sched.append((t, m, m + 1))
                else:
                    dve_sched.append((t, a, b))
        for (t, a, b) in dve_sched:
            nc.vector.tensor_mul(qc[t][:, a * MW:b * MW],
                                 qkt[:, a * MW:b * MW],
                                 _rep(tab[t][:, :], 2 * (b - a)))

        # ---- V projection groups (interleaved into score stream) ----
        vb = [feats.tile([128, MF], BF, name=f"vb{t}") for t in range(4)]
        v_groups = [(t, mg) for t in range(4) for mg in range(3)]
        v_idx = 0

        def emit_v_group():
            nonlocal v_idx
            if v_idx >= len(v_groups):
                return
            t, mg = v_groups[v_idx]
            v_idx += 1
            pv = psV.tile([128, 384], FP, name="psv")
            for i in range(3):
                m = mg * 3 + i
                d = int(DEG_IDX[m])
                nc.tensor.matmul(pv[:, i * 128:(i + 1) * 128],
                                 xT[m][:, t * 128:(t + 1) * 128],
                                 wv_t[:, d * 128:(d + 1) * 128],
                                 start=(i == 0), stop=(i == 2))
            if with_bias and mg == 0:
                nc.vector.tensor_add(vb[t][:, 0:128], pv[:, 0:128], bvb)
                nc.scalar.copy(vb[t][:, 128:384], pv[:, 128:384])
            else:
                nc.scalar.copy(vb[t][:, mg * 384:(mg + 1) * 384], pv)

        # ---- scores: S^T per graph in one PSUM bank, DVE-production order,
        # V groups as filler between blocks ----
        st_ps = [psS.tile([128, 512], FP, name=f"st{g}") for g in range(GPC)]
        started = [False, False]
        total_mm = 6 * NUM_DEG * GPC * 2
        done_mm = 0

        def score_block(t, a, b, gs):
            nonlocal done_mm
            for m in range(a, b):
                base = m * MW
                for g in gs:
                    for h in range(2):
                        done_mm += 1
                        nc.tensor.matmul(
                            st_ps[g][:, h * NG:(h + 1) * NG],
                            qc[t][:, base + NS + g * NG + h * 128:
                                   base + NS + g * NG + h * 128 + 128],
                            qc[t][:, base + g * NG:base + (g + 1) * NG],
                            start=not started[g], stop=done_mm == total_mm)
                        started[g] = True

        st_sb = [stp.tile([128, 512], BF, name=f"stsb{g}") for g in range(GPC)]

        for i, (t, a, b) in enumerate(dve_sched):
            last = i == len(dve_sched) - 1
            if last:
                score_block(t, a, b, [0])
                nc.scalar.copy(st_sb[0], st_ps[0])     # overlaps g1 scores
                score_block(t, a, b, [1])
            else:
                score_block(t, a, b, [0, 1])
                emit_v_group()
                if i % 2 == 0:
                    emit_v_group()
        while v_idx < len(v_groups):
            emit_v_group()

        # ---- tail: outs per graph; DMA each 384-col chunk asap ----
        for g in range(GPC):
            if g == 1:
                nc.scalar.copy(st_sb[1], st_ps[1])
            for mb in range(2):
                qb = g * 2 + mb
                osb = outp.tile([128, MF], BF, name=f"osb{qb}")
                for dc in range(3):
                    po = psQ.tile([128, 384], FP, name="psq")
                    for h in range(2):
                        nc.tensor.matmul(
                            po,
                            st_sb[g][:, h * NG + mb * 128: h * NG + mb * 128 + 128],
                            vb[g * 2 + h][:, dc * 384:(dc + 1) * 384],
                            start=(h == 0), stop=(h == 1))
                    if dc % 2 == 0:
                        nc.scalar.copy(osb[:, dc * 384:(dc + 1) * 384], po)
                    else:
                        nc.vector.tensor_copy(osb[:, dc * 384:(dc + 1) * 384], po)
                    nc.sync.dma_start(
                        out=y[qb * 128:(qb + 1) * 128, dc * 384:(dc + 1) * 384],
                        in_=osb[:, dc * 384:(dc + 1) * 384])


_CACHE = {}


def _get_nc(with_bias):
    if with_bias not in _CACHE:
        _CACHE[with_bias] = _build(with_bias)
    return _CACHE[with_bias]


def make_in_maps(inputs, positions, Wq, bq, Wk, bk, Wv, bv, with_bias):
    import ml_dtypes
    theta = np.linspace(0.0, MAX_FREQ, H, dtype=np.float64) / MAX_LEN
    thdup = np.repeat(theta, 2)                       # (128,)

    # host-precomputed RoPE tables: ang[a, f, n] = thdup[f] * pos[n, a]
    ang = thdup[None, :, None] * positions.T.astype(np.float64)[:, None, :]  # (3,128,N)
    cs = np.empty((6, 128, N), dtype=np.float64)
    cs[0::2] = np.cos(ang)
    cs[1::2] = np.sin(ang)
    cs = cs.astype(ml_dtypes.bfloat16)

    # fold 1/3 quadrature into the Q projection
    wq_h = (Wq.astype(np.float64) / 3.0).transpose(1, 0, 2).reshape(128, 384).astype(ml_dtypes.bfloat16)
    wk_h = Wk.transpose(1, 0, 2).reshape(128, 384).astype(ml_dtypes.bfloat16)
    wv_h = Wv.transpose(1, 0, 2).reshape(128, 384).astype(ml_dtypes.bfloat16)
    x_t = np.ascontiguousarray(inputs.reshape(N, MF).T).astype(ml_dtypes.bfloat16)

    common = dict(wq=wq_h, wk=wk_h, wv=wv_h)
    if with_bias:
        common.update(bq=(bq / 3.0).astype(np.float32).reshape(128, 1),
                      bk=bk.reshape(128, 1).copy(),
                      bvr=bv.reshape(1, 128).copy())
    in_maps = []
    for c in range(N_CORES):
        sl = slice(c * NS, (c + 1) * NS)
        m = dict(common)
        m["xtb"] = np.ascontiguousarray(x_t[:, sl])
        m["tabs"] = np.ascontiguousarray(cs[:, :, sl].reshape(6 * 128, NS))
        in_maps.append(m)
    return in_maps


def kernel(inputs, positions, batch_segments, graph_mask, Wq, bq, Wk, bk, Wv, bv):
    inputs = np.asarray(inputs, dtype=np.float32)
    positions = np.asarray(positions, dtype=np.float32)
    Wq = np.asarray(Wq, dtype=np.float32)
    Wk = np.asarray(Wk, dtype=np.float32)
    Wv = np.asarray(Wv, dtype=np.float32)
    bq = np.asarray(bq, dtype=np.float32)
    bk = np.asarray(bk, dtype=np.float32)
    bv = np.asarray(bv, dtype=np.float32)

    with_bias = bool(np.any(bq) or np.any(bk) or np.any(bv))
    nc = _get_nc(with_bias)
    in_maps = make_in_maps(inputs, positions, Wq, bq, Wk, bk, Wv, bv, with_bias)

    res = run_bass_kernel_spmd(nc, in_maps, core_ids=list(range(N_CORES)))
    out = np.concatenate([np.asarray(r["y"], dtype=np.float32) for r in res.results],
                         axis=0)
    out = out.reshape(N, 1, NUM_DEG, FV)

    mask = np.asarray(graph_mask)[np.asarray(batch_segments)]
    if not mask.all():
        out = out * mask[:, None, None, None].astype(np.float32)
    return out
